# revision 1
# baseline (speedup 1.0000x reference)
"""Multi-head attention (RoPE) Trainium2 kernel.

Problem: B=2, T=2048, D_MODEL=1024, 16 heads x d_k=64, fp32 in/out.

Sharding: tensor-parallel over heads. Core c owns heads 2c, 2c+1:
  - wq/wk/wv rows [128c, 128c+128)  (column-split of the projections)
  - wo columns [128c, 128c+128)     (row-split of the output projection)
Each core emits a fully-normalized fp16 partial of the output projection;
the host sums the 8 partials (the "all-reduce" of row-parallel wo).

On-chip dataflow per core (fp16 matmul operands, fp32 PSUM):
  xT [D=1024, tok=4096] (token-major b*2048+s) @ wT slices -> QT/KT/VT [128, 4096]
  RoPE on QT/KT in [d', tok] layout per 1024-token chunk (tables precomputed
  host-side, partition swap via SBUF-SBUF DMA).
  V transposed per 128-token tile on the PE to per-head [tok, 64] tiles
  (native-F16 PSUM tiles in the shared ring: a bitcast view would hide the
  transpose write from the dependency tracker).
  Scores ST[k, q] = K @ Q^T per head, row-tiled (0,0)/(64,0).
  exp on ScalarE (scale=1/8 folded in; no max-subtraction: scores ~ N(0,1)).
  AV col-packed: head A -> PSUM partitions 0:64 (col groups 0-1), head B ->
  64:128 (col groups 2-3), running concurrently on separate XBUS streams.
  Softmax denominators via four concurrent col-tiled M=1 ones-matmuls into a
  separate accumulator bank. Normalization on device: fast-reciprocal of the
  4 denominator rows, partition-broadcast via K=1 outer-product matmuls,
  ocat = O_unnorm * recip. The two heads' output projections then fuse into
  single K=128 matmuls and one fp16 output tensor.

Schedule: the ScalarE exp stream (64 kt x 2 heads x ~1.15us) and the PE
(~190us of matmul slots) are co-critical; projection/RoPE/transpose/
normalize/oproj work is chopped into "fill units" and interleaved one per
kt slot, deadline-ordered so no matmul ever heads the PE queue before its
producer is emitted (FIFO engines: a queue-head wait on work emitted later
deadlocks or reads garbage). The deferred normalize of chunk c is the FIRST
fill of chunk c+1 (its av-ring free gates av(0)). Input DMAs are issued in
consumption order; Sync-engine dispatch is ~600ns/descriptor, so bulk loads
are single 3D-AP DMAs and later tranches queue behind the latency-critical
rope-swap DMAs.

PSUM budget (8 banks): "big" ring [128,1024]x2 = 4 (scores/proj/transpose/
oproj/recip-broadcast), av [128,512]x2 = 2 (both heads col-packed per query
half), ones [128,512]x2 = 2.
"""

import sys

sys.path.insert(0, "/opt/trn_rl_repo")

import numpy as np

import concourse.bacc as bacc
import concourse.bass as bass
import concourse.tile as tile
from concourse import mybir
from concourse.masks import make_identity

F16 = mybir.dt.float16
F32 = mybir.dt.float32
F8 = mybir.dt.float8e4
DR = mybir.MatmulPerfMode.DoubleRow

B = 2
T = 2048
D = 1024
NTOK = B * T  # 4096
DK = 64
N_CORES = 8
QCH = 1024  # query chunk (per (b, qh))
KT_N = T // 128  # 16 key tiles per batch


def _build_body(tc, x8, wq8, wk8, wv8, woT, ropeA, ropeB, outTA):
    nc = tc.nc
    Exp = mybir.ActivationFunctionType.Exp

    const = tc.alloc_tile_pool(name="const", bufs=1)
    psum = tc.alloc_tile_pool(name="psum", bufs=1, space="PSUM")

    # ---------------- persistent tiles ----------------
    # DMA order = critical path: proj weights, then x/rope tables for t4=0
    # first (the prologue's inputs), then the later t4 tranches, wo last
    # (first needed by oproj units early in C01).
    w_sb = {}
    for nm, w in (("wq", wq8), ("wk", wk8), ("wv", wv8)):
        wt = const.tile([128, 8, 128], F16, name=f"{nm}sb")
        nc.sync.dma_start(out=wt, in_=w.rearrange("(a p) m -> p a m", p=128))
        w_sb[nm] = wt
    ident = const.tile([128, 128], F16)
    make_identity(nc, ident)
    ones_sb = const.tile([128, 64], F16)
    nc.vector.memset(ones_sb, 1.0)

    q_rot = const.tile([128, 4096], F16)
    k_rot = const.tile([128, 4096], F16)
    vt_raw = const.tile([128, 4096], F16)
    # per 128-token tile: [tok, 64] fp16 per head
    v_sb = [
        [const.tile([128, 64], F16, name=f"vsb{i}h{h}") for h in range(2)]
        for i in range(NTOK // 128)
    ]

    at = tc.alloc_tile_pool(name="attn", bufs=1)

    xs = const.tile([128, 8, 4096], F16, name="xs")
    rA = const.tile([128, 4096], F16)
    rB = const.tile([128, 4096], F16)
    wo_sb = const.tile([128, 1024], F16)

    def load_t4(t4):
        # emitted lazily so the big t4=2,3 loads queue BEHIND the prologue's
        # latency-critical rope-swap DMAs on the FIFO sync queue
        cs = slice(t4 * 1024, (t4 + 1) * 1024)
        nc.sync.dma_start(
            out=xs[:, :, cs],
            in_=x8[:, cs].rearrange("(a p) n -> p a n", p=128),
        )
        nc.sync.dma_start(out=rA[:, cs], in_=ropeA[:, cs])
        nc.sync.dma_start(out=rB[:, cs], in_=ropeB[:, cs])
        if t4 == 1:
            nc.sync.dma_start(out=wo_sb, in_=woT)

    load_t4(0)

    # ---------------- phase P building blocks (emitted as fill units) ------
    # Each unit is self-contained (tile alloc + matmuls + eviction) so the
    # shared "big" PSUM ring frees in allocation order without cross-unit
    # stalls. A proj half-unit computes 512 tokens of one projection.
    def proj_half(nm, dst, t4, h2):
        off = t4 * 1024 + h2 * 512
        wt = w_sb[nm]
        ps = psum.tile([128, 1024], F32, tag="big", bufs=2, name="ps_pr")
        for k in range(8):
            nc.tensor.matmul(
                ps[:, 0:512],
                lhsT=wt[:, k, :],
                rhs=xs[:, k, off : off + 512],
                start=(k == 0),
                stop=(k == 7),
            )
        nc.vector.tensor_copy(dst[:, off : off + 512], ps[:, 0:512])

    def rope_unit(raw, t4):
        # out = raw*A + swap(raw)*B, swap = +-32 partitions within a head
        cs = slice(t4 * 1024, (t4 + 1) * 1024)
        sw = at.tile([128, 1024], F16, tag="sw", bufs=2, name="ropesw")
        for dst_p, src_p in ((0, 32), (32, 0), (64, 96), (96, 64)):
            nc.sync.dma_start(
                out=sw[dst_p : dst_p + 32, :], in_=raw[src_p : src_p + 32, cs]
            )
        t1 = at.tile([128, 1024], F16, tag="t1", bufs=2, name="ropet1")
        nc.vector.tensor_mul(t1, raw[:, cs], rA[:, cs])
        nc.vector.tensor_mul(sw, sw, rB[:, cs])
        nc.vector.tensor_add(raw[:, cs], t1, sw)

    def tr_unit(t4, j):
        # V transpose on the PE: vt_raw [d', tok] -> v_sb [tok128, d64] x2.
        # Own F16 tag: a bitcast view on the shared "big" ring hides the
        # transpose write from the overlap tracker (PE clobbers a scores
        # bank mid-ACT-read on the slower cores).
        for i in range(8 * t4 + 2 * j, 8 * t4 + 2 * j + 2):
            ts = slice(i * 128, (i + 1) * 128)
            tr = psum.tile([128, 2048], F16, tag="big", bufs=2, name="ps_tr")
            nc.tensor.transpose(tr[:, 0:128], vt_raw[:, ts], ident)
            nc.vector.tensor_copy(v_sb[i][0], tr[:, 0:64])
            nc.vector.tensor_copy(v_sb[i][1], tr[:, 64:128])

    def phase_units(t4):
        # k first (attention kt tiles need it earliest), then v + transposes,
        # q last (only needed once this t4's own query chunk starts).
        units = [] if t4 not in (1, 2) else [lambda: load_t4(t4 + 1)]
        units += [
            lambda: proj_half("wk", k_rot, t4, 0),
            lambda: (proj_half("wk", k_rot, t4, 1), rope_unit(k_rot, t4)),
            lambda: proj_half("wv", vt_raw, t4, 0),
            lambda: proj_half("wv", vt_raw, t4, 1),
        ]
        for j in range(4):
            units.append(lambda j=j: tr_unit(t4, j))
        units.append(lambda: proj_half("wq", q_rot, t4, 0))
        units.append(lambda: (proj_half("wq", q_rot, t4, 1), rope_unit(q_rot, t4)))
        return units

    # ---------------- attention ----------------
    def chunk(b, qh, fills, pre_av=1):
        """fills: list of callables, ~0.5-2us of PE work each, run one per
        kt slot to keep the PE dense while ScalarE grinds through the exps.
        pre_av: fills to run before av(0) (C00 needs its own V transposed)."""
        qoff = b * T + qh * QCH
        fills = list(fills)

        exp_tiles = {}

        def s_exp(kt):
            koff = b * T + kt * 128
            pss = [
                psum.tile([128, 1024], F32, tag="big", bufs=2, name=f"ps_s{hi}")
                for hi in range(2)
            ]
            for h2 in range(2):
                for hi in range(2):
                    hs = slice(64 * hi, 64 * hi + 64)
                    nc.tensor.matmul(
                        pss[hi][:, h2 * 512 : (h2 + 1) * 512],
                        lhsT=k_rot[hs, koff : koff + 128],
                        rhs=q_rot[hs, qoff + h2 * 512 : qoff + (h2 + 1) * 512],
                        start=True,
                        stop=True,
                    )
            for hi in range(2):
                e = at.tile([128, 1024], F16, tag="exp", bufs=6, name="exps")
                nc.scalar.activation(e, pss[hi], Exp, scale=0.125)
                exp_tiles[(hi, kt)] = e

        ps_av = []
        ps_ones = [None]

        def av(kt):
            vA, vB = v_sb[b * KT_N + kt]
            eA = exp_tiles.pop((0, kt))
            eB = exp_tiles.pop((1, kt))
            st = dict(start=(kt == 0), stop=(kt == KT_N - 1), skip_group_check=True)
            for h2 in range(2):
                h2s = slice(h2 * 512, (h2 + 1) * 512)
                nc.tensor.matmul(ps_av[h2][0:64, :], lhsT=vA, rhs=eA[:, h2s], **st)
                nc.tensor.matmul(ps_av[h2][64:128, :], lhsT=vB, rhs=eB[:, h2s], **st)
            for hi in range(2):
                for h2 in range(2):
                    p = 32 * (2 * hi + h2)
                    e = eA if hi == 0 else eB
                    h2s = slice(h2 * 512, (h2 + 1) * 512)
                    nc.tensor.matmul(
                        ps_ones[0][p : p + 1, :],
                        lhsT=ones_sb[:, 0:1],
                        rhs=e[:, h2s],
                        tile_position=(0, p),
                        **st,
                    )

        # pipelined emission with one-step lag; one fill unit per kt slot
        s_exp(0)
        ps_av.extend(
            psum.tile([128, 512], F32, tag="av", bufs=2, name=f"ps_av{h2}")
            for h2 in range(2)
        )
        ps_ones[0] = psum.tile([128, 512], F32, tag="ones", bufs=2, name="ps_on")
        s_exp(1)
        for _ in range(pre_av):
            if fills:
                fills.pop(0)()
        av(0)
        for kt in range(2, KT_N):
            s_exp(kt)
            av(kt - 1)
            if fills:
                fills.pop(0)()
        av(KT_N - 1)
        while fills:
            fills.pop(0)()

        ocat = at.tile([128, 1024], F16, tag="ocat", bufs=3, name="ocat")

        def norm_unit():
            # normalize on device: reciprocal of the 4 denominator rows,
            # broadcast each across its head's 64 partitions via a K=1
            # outer-product matmul, then ocat = O_unnorm * recip (fp16).
            # With ocat normalized, the two heads' output projections fuse
            # into single K=128 matmuls.
            recip = at.tile([128, 512], F16, tag="recip", bufs=2, name="recip")
            with nc.allow_low_precision(reason="softmax recip in f16"):
                nc.vector.reciprocal(recip, ps_ones[0])
            ps_rb = psum.tile([128, 1024], F32, tag="big", bufs=2, name="ps_rb")
            for hi in range(2):
                for h2 in range(2):
                    p = 32 * (2 * hi + h2)
                    nc.tensor.matmul(
                        ps_rb[64 * hi : 64 * hi + 64, h2 * 512 : (h2 + 1) * 512],
                        lhsT=ones_sb[p : p + 1, :],
                        rhs=recip[p : p + 1, :],
                        start=True,
                        stop=True,
                        tile_position=(p, 64 * hi),
                    )
            rbc = at.tile([128, 1024], F16, tag="rbc", bufs=2, name="rbc")
            nc.vector.tensor_copy(rbc, ps_rb)
            for h2 in range(2):
                h2s = slice(h2 * 512, (h2 + 1) * 512)
                nc.vector.tensor_mul(ocat[:, h2s], ps_av[h2], rbc[:, h2s])

        def oproj_unit(nt):
            nts = slice(nt * 128, (nt + 1) * 128)
            ps_u = psum.tile([128, 1024], F32, tag="big", bufs=2, name="ps_u")
            for h2 in range(2):
                h2s = slice(h2 * 512, (h2 + 1) * 512)
                nc.tensor.matmul(
                    ps_u[:, h2s],
                    lhsT=wo_sb[:, nts],
                    rhs=ocat[:, h2s],
                    start=True,
                    stop=True,
                )
            ot = at.tile([128, 1024], F16, tag="ot", bufs=4, name="ot")
            nc.vector.tensor_copy(ot, ps_u)
            nc.sync.dma_start(out=outTA[nts, qoff : qoff + QCH], in_=ot)

        return [norm_unit] + [lambda nt=nt: oproj_unit(nt) for nt in range(8)]

    # ---------------- schedule ----------------
    # Prologue: only K+Q of t4=0 (attention can start before V is ready).
    # P(t4) units and the deferred normalize/oproj of finished chunks are
    # spread one-per-kt-slot so the PE queue never stalls behind a long
    # dependency chain. Each phase's K/V lands a few slots before the first
    # score/AV matmul that consumes it.
    p0 = phase_units(0)
    p1 = phase_units(1)
    p2 = phase_units(2)
    p3 = phase_units(3)
    # split prologue: K then Q of t4=0 only (with the t4=1 tranche queued
    # between, so the rope-swap DMAs of K don't wait behind it); V-proj and
    # transposes of t4=0 run as C00's first fills, letting ScalarE start on
    # exp(0) ~16us earlier.
    # full P0 prologue; the t4=1 input tranche is queued right after the
    # K-rope swaps so the Q-rope swap DMAs (which gate the first scores ->
    # first exp) aren't stuck behind 3.25MB of loads.
    p0[0]()
    p0[1]()
    load_t4(1)
    for u in p0[2:]:
        u()
    n00 = chunk(0, 0, p1, pre_av=1)
    # norm(c) must be the first fill of chunk c+1, BEFORE its av(0): av(0)
    # waits on the av-ring freed by norm's DVE ops, and any fill copy queued
    # between them on the DVE would wait on PE work behind the stalled av(0).
    n01 = chunk(0, 1, [n00[0]] + p2 + n00[1:5])
    n10 = chunk(1, 0, [n01[0]] + p3 + n00[5:9])
    n11 = chunk(1, 1, [n10[0]] + n01[1:9] + n10[1:7])
    for u in n10[7:9] + n11:
        u()

    at.release()
    const.release()
    psum.release()


def _interleave(a, b):
    out = []
    la, lb = list(a), list(b)
    while la or lb:
        if la:
            out.append(la.pop(0))
        if lb:
            out.append(lb.pop(0))
    return out


_NC_CACHE = {}


def _build_program():
    if 0 in _NC_CACHE:
        return _NC_CACHE[0]
    nc = bacc.Bacc("TRN2", num_devices=N_CORES, debug=False)
    x8 = nc.dram_tensor("xT", [D, NTOK], F16, kind="ExternalInput").ap()
    wq8 = nc.dram_tensor("wqT", [D, 128], F16, kind="ExternalInput").ap()
    wk8 = nc.dram_tensor("wkT", [D, 128], F16, kind="ExternalInput").ap()
    wv8 = nc.dram_tensor("wvT", [D, 128], F16, kind="ExternalInput").ap()
    woT = nc.dram_tensor("woT", [128, D], F16, kind="ExternalInput").ap()
    ropeA = nc.dram_tensor("ropeA", [128, NTOK], F16, kind="ExternalInput").ap()
    ropeB = nc.dram_tensor("ropeB", [128, NTOK], F16, kind="ExternalInput").ap()
    outTA = nc.dram_tensor("outTA", [D, NTOK], F16, kind="ExternalOutput").ap()
    with tile.TileContext(nc) as tc:
        _build_body(tc, x8, wq8, wk8, wv8, woT, ropeA, ropeB, outTA)
    nc.compile()
    _NC_CACHE[0] = nc
    return nc


def _rope_tables():
    half = DK // 2  # 32
    inv_freq = 1.0 / (
        10000.0 ** (np.arange(0, DK, 2, dtype=np.float32) / np.float32(DK))
    )
    t = np.arange(T, dtype=np.float32)
    freqs = np.outer(t, inv_freq)  # [T, 32]
    cos = np.cos(freqs)
    sin = np.sin(freqs)
    A = np.empty((128, NTOK), np.float32)
    Bt = np.empty((128, NTOK), np.float32)
    for p in range(128):
        i = p % DK
        if i < half:
            a, bb = cos[:, i], -sin[:, i]
        else:
            a, bb = cos[:, i - half], sin[:, i - half]
        for bi in range(B):
            A[p, bi * T : (bi + 1) * T] = a
            Bt[p, bi * T : (bi + 1) * T] = bb
    return A.astype(np.float16), Bt.astype(np.float16)


def _prep_inputs(x, wq, wk, wv, wo):
    xT = np.ascontiguousarray(x.reshape(NTOK, D).T).astype(np.float16)
    ropeA, ropeB = _rope_tables()
    in_maps = []
    for c in range(N_CORES):
        rows = slice(128 * c, 128 * (c + 1))
        in_maps.append(
            {
                "xT": xT,
                "wqT": np.ascontiguousarray(wq[rows, :].T).astype(np.float16),
                "wkT": np.ascontiguousarray(wk[rows, :].T).astype(np.float16),
                "wvT": np.ascontiguousarray(wv[rows, :].T).astype(np.float16),
                "woT": np.ascontiguousarray(wo[:, rows].T).astype(np.float16),
                "ropeA": ropeA,
                "ropeB": ropeB,
            }
        )
    return in_maps


def run(x, wq, wk, wv, wo, trace=False):
    """Returns (output (B,T,D) fp32, BassKernelResults)."""
    from concourse import bass_utils

    nc = _build_program()
    in_maps = _prep_inputs(
        np.asarray(x, np.float32),
        np.asarray(wq, np.float32),
        np.asarray(wk, np.float32),
        np.asarray(wv, np.float32),
        np.asarray(wo, np.float32),
    )
    res = bass_utils.run_bass_kernel_spmd(
        nc, in_maps, core_ids=list(range(N_CORES)), trace=trace
    )
    acc = np.zeros((D, NTOK), np.float32)
    for c in range(N_CORES):
        acc += np.asarray(res.results[c]["outTA"], np.float32)
    out = acc.T.reshape(B, T, D)
    return out, res


def kernel(x, wq, wk, wv, wo):
    out, _ = run(x, wq, wk, wv, wo)
    return out



# revision 2
# speedup vs baseline: 1.3043x; 1.3043x over previous
"""Multi-head attention (RoPE) Trainium2 kernel — v2.

Problem: B=2, T=2048, D_MODEL=1024, 16 heads x d_k=64, fp32 in/out.

Sharding: tensor-parallel over heads. Core c owns heads 2c, 2c+1:
  - wq/wk/wv rows [128c, 128c+128)  (column-split of the projections)
  - wo columns [128c, 128c+128)     (row-split of the output projection)
Each core emits a fully-normalized fp16 partial of the output projection;
the host sums the 8 partials (the "all-reduce" of row-parallel wo).

v2 changes vs the 301us baseline (trace-driven):
  - RoPE pair layout permuted host-side so the rotate-half partner of every
    row lives in the same 32-partition quadrant: the partition swap becomes
    one DVE stream_shuffle instead of 4 latency-critical SBUF-SBUF DMAs.
    (Scores are invariant to a common within-head permutation of Q/K rows.)
  - V transpose moved off the PE onto the DMA XBAR (dma_start_transpose,
    [128,128] f16 tiles) - frees ~12us of PE timeline + 64 DVE copies.
  - softmax reciprocal via reciprocal_approx_fast (5x) + f16 cast; the
    partition-broadcast stays on the PE (4-way concurrent M=1 matmuls
    measured at ~57ns/MM).
  - all x/rope-table tranches DMA'd up front (the sync queue no longer
    carries rope swaps, so nothing latency-critical queues behind them).
  - ACT exp table preloaded via a dummy activation at kernel start (hides
    the ~2.7us ACT_TABLE_LOAD under the prologue DMA wait).
  - V projection of t4=0 runs as the first fills of chunk 0 (PE does V-proj
    while DVE ropes Q), prologue = K-proj+rope, Q-proj+rope only.

Schedule: per kt slot the PE runs 4 score MMs (M=128, serial, ~0.9us) and
4 AV MMs + 4 ones MMs (M=64 pairs + 4-way M=1, ~0.9us) against an ACT
envelope of 2x 1147ns exps; fill units (proj halves / oproj / norm) are
interleaved one per slot into the remaining slack, deadline-ordered so no
matmul heads the PE queue before its producer is emitted.

PSUM (8 banks): big ring [128,1024]x2 = 4 (scores/proj/oproj/recip-bcast),
av [128,512]x2 = 2, ones [128,512]x2 = 2.
"""

import sys

sys.path.insert(0, "/opt/trn_rl_repo")

import numpy as np

import concourse.bacc as bacc
import concourse.bass as bass
import concourse.tile as tile
from concourse import mybir

F16 = mybir.dt.float16
F32 = mybir.dt.float32

B = 2
T = 2048
D = 1024
NTOK = B * T  # 4096
DK = 64
N_CORES = 8
QCH = 1024  # query chunk (per (b, qh))
KT_N = T // 128  # 16 key tiles per batch

SWAP_MASK = list(range(16, 32)) + list(range(0, 16))


def _build_body(tc, x8, wq8, wk8, wv8, woT, ropeA, ropeB, outTA):
    nc = tc.nc
    Exp = mybir.ActivationFunctionType.Exp

    const = tc.alloc_tile_pool(name="const", bufs=1)
    psum = tc.alloc_tile_pool(name="psum", bufs=1, space="PSUM")

    # ---------------- persistent tiles ----------------
    # DMA order = consumption order: wk (first PE work), x/rope t4=0, wq,
    # then the remaining tranches up front (nothing latency-critical queues
    # behind them anymore), wv, wo.
    w_sb = {}
    for nm, w in (("wk", wk8), ("wq", wq8), ("wv", wv8)):
        wt = const.tile([128, 8, 128], F16, name=f"{nm}sb")
        w_sb[nm] = wt

    xs = const.tile([128, 8, 4096], F16, name="xs")
    rA = const.tile([128, 4096], F16)
    rB = const.tile([128, 4096], F16)
    wo_sb = const.tile([128, 1024], F16)

    nc.sync.dma_start(
        out=w_sb["wk"], in_=wk8.rearrange("(a p) m -> p a m", p=128)
    )

    def load_t4(t4):
        cs = slice(t4 * 1024, (t4 + 1) * 1024)
        nc.sync.dma_start(
            out=xs[:, :, cs],
            in_=x8[:, cs].rearrange("(a p) n -> p a n", p=128),
        )
        nc.sync.dma_start(out=rA[:, cs], in_=ropeA[:, cs])
        nc.sync.dma_start(out=rB[:, cs], in_=ropeB[:, cs])

    load_t4(0)
    nc.sync.dma_start(
        out=w_sb["wq"], in_=wq8.rearrange("(a p) m -> p a m", p=128)
    )
    load_t4(1)
    nc.sync.dma_start(
        out=w_sb["wv"], in_=wv8.rearrange("(a p) m -> p a m", p=128)
    )
    load_t4(2)
    nc.sync.dma_start(out=wo_sb, in_=woT)
    load_t4(3)

    ones_sb = const.tile([128, 64], F16)
    nc.vector.memset(ones_sb, 1.0)

    q_rot = const.tile([128, 4096], F16)
    k_rot = const.tile([128, 4096], F16)
    vt_raw = const.tile([128, 4096], F16)
    # per 128-token tile: [tok, d'] fp16, both heads (A cols 0:64, B 64:128)
    v_sb = [const.tile([128, 128], F16, name=f"vsb{i}") for i in range(NTOK // 128)]

    at = tc.alloc_tile_pool(name="attn", bufs=1)

    # preload the exp table set under the prologue DMA wait
    warm = at.tile([128, 1], F16, tag="warm", bufs=1)
    nc.scalar.activation(warm, ones_sb[:, 0:1], Exp, scale=0.125)

    # ---------------- fill-unit building blocks ----------------
    def proj_half(nm, dst, t4, h2):
        off = t4 * 1024 + h2 * 512
        wt = w_sb[nm]
        ps = psum.tile([128, 1024], F32, tag="big", bufs=2, name="ps_pr")
        for k in range(8):
            nc.tensor.matmul(
                ps[:, 0:512],
                lhsT=wt[:, k, :],
                rhs=xs[:, k, off : off + 512],
                start=(k == 0),
                stop=(k == 7),
            )
        nc.vector.tensor_copy(dst[:, off : off + 512], ps[:, 0:512])
        if nm == "wv":
            # V^T -> per-token-tile [tok, d'] via the DMA XBAR transpose
            for i in range(off // 128, off // 128 + 4):
                ts = slice(i * 128, (i + 1) * 128)
                nc.sync.dma_start_transpose(out=v_sb[i], in_=vt_raw[:, ts])

    def rope_unit(raw, t4):
        # out = raw*A + shuffle(raw)*B; the rotate-half partner of every row
        # is +-16 within its 32-partition quadrant (host-permuted layout)
        cs = slice(t4 * 1024, (t4 + 1) * 1024)
        sw = at.tile([128, 1024], F16, tag="sw", bufs=2, name="ropesw")
        nc.vector.stream_shuffle(sw, raw[:, cs], SWAP_MASK)
        t1 = at.tile([128, 1024], F16, tag="t1", bufs=2, name="ropet1")
        nc.vector.tensor_mul(t1, raw[:, cs], rA[:, cs])
        nc.vector.tensor_mul(sw, sw, rB[:, cs])
        nc.vector.tensor_add(raw[:, cs], t1, sw)

    def phase_units(t4):
        # k first (attention kt tiles need it earliest), then v + transposes,
        # q last (only needed once this t4's own query chunk starts).
        return [
            lambda: proj_half("wk", k_rot, t4, 0),
            lambda: (proj_half("wk", k_rot, t4, 1), rope_unit(k_rot, t4)),
            lambda: proj_half("wv", vt_raw, t4, 0),
            lambda: proj_half("wv", vt_raw, t4, 1),
            lambda: proj_half("wq", q_rot, t4, 0),
            lambda: (proj_half("wq", q_rot, t4, 1), rope_unit(q_rot, t4)),
        ]

    # ---------------- attention ----------------
    def chunk(b, qh, fills, pre_av=1):
        """fills: callables, ~0.5-2us of PE work each, one per kt slot.
        pre_av: fills to run before av(0) (C00 needs its own V ready)."""
        qoff = b * T + qh * QCH
        fills = list(fills)

        exp_tiles = {}

        def s_exp(kt):
            koff = b * T + kt * 128
            pss = [
                psum.tile([128, 1024], F32, tag="big", bufs=2, name=f"ps_s{hi}")
                for hi in range(2)
            ]
            for h2 in range(2):
                for hi in range(2):
                    hs = slice(64 * hi, 64 * hi + 64)
                    nc.tensor.matmul(
                        pss[hi][:, h2 * 512 : (h2 + 1) * 512],
                        lhsT=k_rot[hs, koff : koff + 128],
                        rhs=q_rot[hs, qoff + h2 * 512 : qoff + (h2 + 1) * 512],
                        start=True,
                        stop=True,
                    )
            for hi in range(2):
                e = at.tile([128, 1024], F16, tag="exp", bufs=6, name="exps")
                nc.scalar.activation(e, pss[hi], Exp, scale=0.125)
                exp_tiles[(hi, kt)] = e

        ps_av = []
        ps_ones = [None]

        def av(kt):
            vt = v_sb[b * KT_N + kt]
            eA = exp_tiles.pop((0, kt))
            eB = exp_tiles.pop((1, kt))
            st = dict(start=(kt == 0), stop=(kt == KT_N - 1), skip_group_check=True)
            for h2 in range(2):
                h2s = slice(h2 * 512, (h2 + 1) * 512)
                nc.tensor.matmul(
                    ps_av[h2][0:64, :], lhsT=vt[:, 0:64], rhs=eA[:, h2s], **st
                )
                nc.tensor.matmul(
                    ps_av[h2][64:128, :], lhsT=vt[:, 64:128], rhs=eB[:, h2s], **st
                )
            for hi in range(2):
                for h2 in range(2):
                    p = 32 * (2 * hi + h2)
                    e = eA if hi == 0 else eB
                    h2s = slice(h2 * 512, (h2 + 1) * 512)
                    nc.tensor.matmul(
                        ps_ones[0][p : p + 1, :],
                        lhsT=ones_sb[:, 0:1],
                        rhs=e[:, h2s],
                        tile_position=(0, p),
                        **st,
                    )

        # pipelined emission with one-step lag; one fill unit per kt slot
        s_exp(0)
        ps_av.extend(
            psum.tile([128, 512], F32, tag="av", bufs=2, name=f"ps_av{h2}")
            for h2 in range(2)
        )
        ps_ones[0] = psum.tile([128, 512], F32, tag="ones", bufs=2, name="ps_on")
        s_exp(1)
        for _ in range(pre_av):
            if fills:
                fills.pop(0)()
        av(0)
        for kt in range(2, KT_N):
            s_exp(kt)
            av(kt - 1)
            if fills:
                fills.pop(0)()
        av(KT_N - 1)
        while fills:
            fills.pop(0)()

        ocat = at.tile([128, 1024], F16, tag="ocat", bufs=3, name="ocat")

        def norm_unit():
            # reciprocal of the 4 denominator rows (fast-approx, ~5x), cast
            # to f16, partition-broadcast via K=1 outer-product matmuls
            # (4-way concurrent), then ocat = O_unnorm * recip.
            r32 = at.tile([128, 512], F32, tag="r32", bufs=2, name="r32")
            nc.vector.reciprocal_approx_fast(r32, ps_ones[0])
            recip = at.tile([128, 512], F16, tag="recip", bufs=2, name="recip")
            nc.vector.tensor_copy(recip, r32)
            ps_rb = psum.tile([128, 1024], F32, tag="big", bufs=2, name="ps_rb")
            for hi in range(2):
                for h2 in range(2):
                    p = 32 * (2 * hi + h2)
                    nc.tensor.matmul(
                        ps_rb[64 * hi : 64 * hi + 64, h2 * 512 : (h2 + 1) * 512],
                        lhsT=ones_sb[p : p + 1, :],
                        rhs=recip[p : p + 1, :],
                        start=True,
                        stop=True,
                        tile_position=(p, 64 * hi),
                    )
            rbc = at.tile([128, 1024], F16, tag="rbc", bufs=2, name="rbc")
            nc.vector.tensor_copy(rbc, ps_rb)
            for h2 in range(2):
                h2s = slice(h2 * 512, (h2 + 1) * 512)
                nc.vector.tensor_mul(ocat[:, h2s], ps_av[h2], rbc[:, h2s])

        def oproj_unit(nt):
            nts = slice(nt * 128, (nt + 1) * 128)
            ps_u = psum.tile([128, 1024], F32, tag="big", bufs=2, name="ps_u")
            for h2 in range(2):
                h2s = slice(h2 * 512, (h2 + 1) * 512)
                nc.tensor.matmul(
                    ps_u[:, h2s],
                    lhsT=wo_sb[:, nts],
                    rhs=ocat[:, h2s],
                    start=True,
                    stop=True,
                )
            ot = at.tile([128, 1024], F16, tag="ot", bufs=4, name="ot")
            nc.vector.tensor_copy(ot, ps_u)
            nc.sync.dma_start(out=outTA[nts, qoff : qoff + QCH], in_=ot)

        return [norm_unit] + [lambda nt=nt: oproj_unit(nt) for nt in range(8)]

    # ---------------- schedule ----------------
    # Prologue: K-proj+rope and Q-proj+rope of t4=0 only; V of t4=0 runs as
    # chunk 0's first fills (PE does V-proj while DVE ropes Q). norm(c) must
    # be the first fill of chunk c+1, BEFORE its av(0): av(0) waits on the
    # av-ring freed by norm's DVE ops (FIFO engines deadlock otherwise).
    p0 = phase_units(0)
    p1 = phase_units(1)
    p2 = phase_units(2)
    p3 = phase_units(3)
    p0[0]()  # K proj half 0
    p0[1]()  # K proj half 1 + rope
    p0[4]()  # Q proj half 0
    p0[5]()  # Q proj half 1 + rope
    n00 = chunk(0, 0, [p0[2], p0[3]] + p1, pre_av=2)
    n01 = chunk(0, 1, [n00[0]] + p2 + n00[1:5])
    n10 = chunk(1, 0, [n01[0]] + p3 + n00[5:9])
    n11 = chunk(1, 1, [n10[0]] + n01[1:9] + n10[1:7])
    for u in n10[7:9] + n11:
        u()

    at.release()
    const.release()
    psum.release()


_NC_CACHE = {}


def _build_program():
    if 0 in _NC_CACHE:
        return _NC_CACHE[0]
    nc = bacc.Bacc("TRN2", num_devices=N_CORES, debug=False)
    x8 = nc.dram_tensor("xT", [D, NTOK], F16, kind="ExternalInput").ap()
    wq8 = nc.dram_tensor("wqT", [D, 128], F16, kind="ExternalInput").ap()
    wk8 = nc.dram_tensor("wkT", [D, 128], F16, kind="ExternalInput").ap()
    wv8 = nc.dram_tensor("wvT", [D, 128], F16, kind="ExternalInput").ap()
    woT = nc.dram_tensor("woT", [128, D], F16, kind="ExternalInput").ap()
    ropeA = nc.dram_tensor("ropeA", [128, NTOK], F16, kind="ExternalInput").ap()
    ropeB = nc.dram_tensor("ropeB", [128, NTOK], F16, kind="ExternalInput").ap()
    outTA = nc.dram_tensor("outTA", [D, NTOK], F16, kind="ExternalOutput").ap()
    with tile.TileContext(nc) as tc:
        _build_body(tc, x8, wq8, wk8, wv8, woT, ropeA, ropeB, outTA)
    nc.compile()
    _NC_CACHE[0] = nc
    return nc


def _rope_perm():
    """Within-head row permutation: new row 32q+j holds old dim 16q+j
    (x1, j<16) or 32+16q+(j-16) (x2, j>=16) so rotate-half partners sit
    +-16 apart inside one 32-partition quadrant."""
    perm = np.empty(DK, np.int64)
    for q in range(2):
        for j in range(32):
            r = 32 * q + j
            perm[r] = 16 * q + j if j < 16 else 32 + 16 * q + (j - 16)
    return perm  # new_row -> old_dim


def _rope_tables():
    half = DK // 2  # 32
    inv_freq = 1.0 / (
        10000.0 ** (np.arange(0, DK, 2, dtype=np.float32) / np.float32(DK))
    )
    t = np.arange(T, dtype=np.float32)
    freqs = np.outer(t, inv_freq)  # [T, 32]
    cos = np.cos(freqs)
    sin = np.sin(freqs)
    perm = _rope_perm()
    A = np.empty((128, NTOK), np.float32)
    Bt = np.empty((128, NTOK), np.float32)
    for p in range(128):
        d = perm[p % DK]
        if d < half:
            a, bb = cos[:, d], -sin[:, d]
        else:
            a, bb = cos[:, d - half], sin[:, d - half]
        for bi in range(B):
            A[p, bi * T : (bi + 1) * T] = a
            Bt[p, bi * T : (bi + 1) * T] = bb
    return A.astype(np.float16), Bt.astype(np.float16)


def _prep_inputs(x, wq, wk, wv, wo):
    xT = np.ascontiguousarray(x.reshape(NTOK, D).T).astype(np.float16)
    ropeA, ropeB = _rope_tables()
    perm = _rope_perm()
    full_perm = np.concatenate([64 * h + perm for h in range(2)])  # 128 rows
    in_maps = []
    for c in range(N_CORES):
        rows = slice(128 * c, 128 * (c + 1))
        wq_c = wq[rows, :][full_perm, :]
        wk_c = wk[rows, :][full_perm, :]
        in_maps.append(
            {
                "xT": xT,
                "wqT": np.ascontiguousarray(wq_c.T).astype(np.float16),
                "wkT": np.ascontiguousarray(wk_c.T).astype(np.float16),
                "wvT": np.ascontiguousarray(wv[rows, :].T).astype(np.float16),
                "woT": np.ascontiguousarray(wo[:, rows].T).astype(np.float16),
                "ropeA": ropeA,
                "ropeB": ropeB,
            }
        )
    return in_maps


def run(x, wq, wk, wv, wo, trace=False):
    """Returns (output (B,T,D) fp32, BassKernelResults)."""
    from concourse import bass_utils

    nc = _build_program()
    in_maps = _prep_inputs(
        np.asarray(x, np.float32),
        np.asarray(wq, np.float32),
        np.asarray(wk, np.float32),
        np.asarray(wv, np.float32),
        np.asarray(wo, np.float32),
    )
    res = bass_utils.run_bass_kernel_spmd(
        nc, in_maps, core_ids=list(range(N_CORES)), trace=trace
    )
    acc = np.zeros((D, NTOK), np.float32)
    for c in range(N_CORES):
        acc += np.asarray(res.results[c]["outTA"], np.float32)
    out = acc.T.reshape(B, T, D)
    return out, res


def kernel(x, wq, wk, wv, wo):
    out, _ = run(x, wq, wk, wv, wo)
    return out


# revision 7
# speedup vs baseline: 1.3272x; 1.0176x over previous
"""Multi-head attention (RoPE) Trainium2 kernel — v2.

Problem: B=2, T=2048, D_MODEL=1024, 16 heads x d_k=64, fp32 in/out.

Sharding: tensor-parallel over heads. Core c owns heads 2c, 2c+1:
  - wq/wk/wv rows [128c, 128c+128)  (column-split of the projections)
  - wo columns [128c, 128c+128)     (row-split of the output projection)
Each core emits a fully-normalized fp16 partial of the output projection;
the host sums the 8 partials (the "all-reduce" of row-parallel wo).

v2 changes vs the 301us baseline (trace-driven):
  - RoPE pair layout permuted host-side so the rotate-half partner of every
    row lives in the same 32-partition quadrant: the partition swap becomes
    one DVE stream_shuffle instead of 4 latency-critical SBUF-SBUF DMAs.
    (Scores are invariant to a common within-head permutation of Q/K rows.)
  - V transpose moved off the PE onto the DMA XBAR (dma_start_transpose,
    [128,128] f16 tiles) - frees ~12us of PE timeline + 64 DVE copies.
  - softmax reciprocal via reciprocal_approx_fast (5x) + f16 cast; the
    partition-broadcast stays on the PE (4-way concurrent M=1 matmuls
    measured at ~57ns/MM).
  - all x/rope-table tranches DMA'd up front (the sync queue no longer
    carries rope swaps, so nothing latency-critical queues behind them).
  - ACT exp table preloaded via a dummy activation at kernel start (hides
    the ~2.7us ACT_TABLE_LOAD under the prologue DMA wait).
  - V projection of t4=0 runs as the first fills of chunk 0 (PE does V-proj
    while DVE ropes Q), prologue = K-proj+rope, Q-proj+rope only.

Schedule: per kt slot the PE runs 4 score MMs (M=128, serial, ~0.9us) and
4 AV MMs + 4 ones MMs (M=64 pairs + 4-way M=1, ~0.9us) against an ACT
envelope of 2x 1147ns exps; fill units (proj halves / oproj / norm) are
interleaved one per slot into the remaining slack, deadline-ordered so no
matmul heads the PE queue before its producer is emitted.

PSUM (8 banks): big ring [128,1024]x2 = 4 (scores/proj/oproj/recip-bcast),
av [128,512]x2 = 2, ones [128,512]x2 = 2.
"""

import sys

sys.path.insert(0, "/opt/trn_rl_repo")

import numpy as np

import concourse.bacc as bacc
import concourse.bass as bass
import concourse.tile as tile
from concourse import mybir

F16 = mybir.dt.float16
F32 = mybir.dt.float32

B = 2
T = 2048
D = 1024
NTOK = B * T  # 4096
DK = 64
N_CORES = 8
QCH = 1024  # query chunk (per (b, qh))
KT_N = T // 128  # 16 key tiles per batch

SWAP_MASK = list(range(16, 32)) + list(range(0, 16))


def _build_body(tc, x8, wq8, wk8, wv8, woT, ropeA, ropeB, outTA):
    nc = tc.nc
    Exp = mybir.ActivationFunctionType.Exp

    const = tc.alloc_tile_pool(name="const", bufs=1)
    psum = tc.alloc_tile_pool(name="psum", bufs=1, space="PSUM")

    # ---------------- persistent tiles ----------------
    # DMA order = consumption order: wk (first PE work), x/rope t4=0, wq,
    # then the remaining tranches up front (nothing latency-critical queues
    # behind them anymore), wv, wo.
    w_sb = {}
    for nm, w in (("wk", wk8), ("wq", wq8), ("wv", wv8)):
        wt = const.tile([128, 8, 128], F16, name=f"{nm}sb")
        w_sb[nm] = wt

    xs = const.tile([128, 8, 4096], F16, name="xs")
    rA = const.tile([128, 4096], F16)
    rB = const.tile([128, 4096], F16)
    wo_sb = const.tile([128, 1024], F16)

    nc.sync.dma_start(
        out=w_sb["wk"], in_=wk8.rearrange("(a p) m -> p a m", p=128)
    )

    def load_t4(t4):
        cs = slice(t4 * 1024, (t4 + 1) * 1024)
        nc.sync.dma_start(
            out=xs[:, :, cs],
            in_=x8[:, cs].rearrange("(a p) n -> p a n", p=128),
        )
        nc.sync.dma_start(out=rA[:, cs], in_=ropeA[:, cs])
        nc.sync.dma_start(out=rB[:, cs], in_=ropeB[:, cs])

    load_t4(0)
    nc.sync.dma_start(
        out=w_sb["wq"], in_=wq8.rearrange("(a p) m -> p a m", p=128)
    )
    load_t4(1)
    nc.sync.dma_start(
        out=w_sb["wv"], in_=wv8.rearrange("(a p) m -> p a m", p=128)
    )
    load_t4(2)
    nc.sync.dma_start(out=wo_sb, in_=woT)
    load_t4(3)

    ones_sb = const.tile([128, 64], F16)
    nc.vector.memset(ones_sb, 1.0)

    q_rot = const.tile([128, 4096], F16)
    k_rot = const.tile([128, 4096], F16)
    vt_raw = const.tile([128, 4096], F16)
    # per 128-token tile: [tok, d'] fp16, both heads (A cols 0:64, B 64:128)
    v_sb = [const.tile([128, 128], F16, name=f"vsb{i}") for i in range(NTOK // 128)]

    at = tc.alloc_tile_pool(name="attn", bufs=1)

    # preload the exp table set under the prologue DMA wait
    warm = at.tile([128, 1], F16, tag="warm", bufs=1)
    nc.scalar.activation(warm, ones_sb[:, 0:1], Exp, scale=0.125)

    # ---------------- fill-unit building blocks ----------------
    # fills use a dedicated 1-bank psum ring so they never wait on the
    # ACT-paced scores ring
    def proj_half(nm, dst, t4, h2):
        off = t4 * 1024 + h2 * 512
        wt = w_sb[nm]
        ps = psum.tile([128, 512], F32, tag="fl", bufs=1, name="ps_pr")
        for k in range(8):
            nc.tensor.matmul(
                ps,
                lhsT=wt[:, k, :],
                rhs=xs[:, k, off : off + 512],
                start=(k == 0),
                stop=(k == 7),
            )
        nc.vector.tensor_copy(dst[:, off : off + 512], ps)
        if nm == "wv":
            # V^T -> per-token-tile [tok, d'] via the DMA XBAR transpose
            for i in range(off // 128, off // 128 + 4):
                ts = slice(i * 128, (i + 1) * 128)
                nc.sync.dma_start_transpose(out=v_sb[i], in_=vt_raw[:, ts])

    def rope_unit(raw, t4):
        # out = raw*A + shuffle(raw)*B; the rotate-half partner of every row
        # is +-16 within its 32-partition quadrant (host-permuted layout)
        cs = slice(t4 * 1024, (t4 + 1) * 1024)
        sw = at.tile([128, 1024], F16, tag="sw", bufs=2, name="ropesw")
        nc.vector.stream_shuffle(sw, raw[:, cs], SWAP_MASK)
        t1 = at.tile([128, 1024], F16, tag="t1", bufs=2, name="ropet1")
        nc.vector.tensor_mul(t1, raw[:, cs], rA[:, cs])
        nc.vector.tensor_mul(sw, sw, rB[:, cs])
        nc.vector.tensor_add(raw[:, cs], t1, sw)

    def phase_units(t4):
        # k first (attention kt tiles need it earliest), then v + transposes,
        # q last (only needed once this t4's own query chunk starts).
        return [
            lambda: proj_half("wk", k_rot, t4, 0),
            lambda: (proj_half("wk", k_rot, t4, 1), rope_unit(k_rot, t4)),
            lambda: proj_half("wv", vt_raw, t4, 0),
            lambda: proj_half("wv", vt_raw, t4, 1),
            lambda: proj_half("wq", q_rot, t4, 0),
            lambda: (proj_half("wq", q_rot, t4, 1), rope_unit(q_rot, t4)),
        ]

    # ---------------- attention ----------------
    def chunk(b, qh, fills, pre_av=1):
        """fills: callables, ~0.5-2us of PE work each, one per kt slot.
        pre_av: fills to run before av(0) (C00 needs its own V ready)."""
        qoff = b * T + qh * QCH
        fills = list(fills)

        exp_tiles = {}

        def s_exp(kt):
            # hi-major: the hi tile's 2 MMs run back-to-back so ACT's exp of
            # head hi starts after 2 MMs, and the per-hi ring slot frees
            # independently (PE's hi0 MMs of kt overlap ACT's hi1 of kt-1).
            koff = b * T + kt * 128
            for hi in range(2):
                ps = psum.tile([128, 1024], F32, tag="sc", bufs=2, name="ps_s")
                hs = slice(64 * hi, 64 * hi + 64)
                for h2 in range(2):
                    nc.tensor.matmul(
                        ps[:, h2 * 512 : (h2 + 1) * 512],
                        lhsT=k_rot[hs, koff : koff + 128],
                        rhs=q_rot[hs, qoff + h2 * 512 : qoff + (h2 + 1) * 512],
                        start=True,
                        stop=True,
                    )
                e = at.tile([128, 1024], F16, tag="exp", bufs=8, name="exps")
                nc.scalar.activation(e, ps, Exp, scale=0.125)
                exp_tiles[(hi, kt)] = e

        ps_av = []
        ps_ones = [None]

        def av(kt):
            vt = v_sb[b * KT_N + kt]
            eA = exp_tiles.pop((0, kt))
            eB = exp_tiles.pop((1, kt))
            st = dict(start=(kt == 0), stop=(kt == KT_N - 1), skip_group_check=True)
            for h2 in range(2):
                h2s = slice(h2 * 512, (h2 + 1) * 512)
                nc.tensor.matmul(
                    ps_av[h2][0:64, :], lhsT=vt[:, 0:64], rhs=eA[:, h2s], **st
                )
                nc.tensor.matmul(
                    ps_av[h2][64:128, :], lhsT=vt[:, 64:128], rhs=eB[:, h2s], **st
                )
            for hi in range(2):
                for h2 in range(2):
                    p = 32 * (2 * hi + h2)
                    e = eA if hi == 0 else eB
                    h2s = slice(h2 * 512, (h2 + 1) * 512)
                    nc.tensor.matmul(
                        ps_ones[0][p : p + 1, :],
                        lhsT=ones_sb[:, 0:1],
                        rhs=e[:, h2s],
                        tile_position=(0, p),
                        **st,
                    )

        # pipelined emission with TWO-slot av lag (exp tiles always banked
        # when av issues -> no LDW stalls on ACT); one fill unit per kt slot
        s_exp(0)
        ps_av.extend(
            psum.tile([128, 512], F32, tag="av", bufs=2, name=f"ps_av{h2}")
            for h2 in range(2)
        )
        ps_ones[0] = psum.tile([128, 512], F32, tag="ones", bufs=1, name="ps_on")
        s_exp(1)
        if fills:
            fills.pop(0)()
        s_exp(2)
        for _ in range(max(0, pre_av - 1)):
            if fills:
                fills.pop(0)()
        av(0)
        for kt in range(3, KT_N):
            s_exp(kt)
            av(kt - 2)
            if fills:
                fills.pop(0)()
        av(KT_N - 2)
        if fills:
            fills.pop(0)()
        av(KT_N - 1)
        while fills:
            fills.pop(0)()

        ocat = at.tile([128, 1024], F16, tag="ocat", bufs=3, name="ocat")

        def norm_unit():
            # reciprocal of the 4 denominator rows (fast-approx, ~5x), cast
            # to f16, partition-broadcast via K=1 outer-product matmuls
            # (4-way concurrent), then ocat = O_unnorm * recip.
            r32 = at.tile([128, 512], F32, tag="r32", bufs=2, name="r32")
            nc.vector.reciprocal_approx_fast(r32, ps_ones[0])
            recip = at.tile([128, 512], F16, tag="recip", bufs=2, name="recip")
            nc.vector.tensor_copy(recip, r32)
            rbc = at.tile([128, 1024], F16, tag="rbc", bufs=2, name="rbc")
            for h2 in range(2):
                ps_rb = psum.tile([128, 512], F32, tag="fl", bufs=1, name="ps_rb")
                for hi in range(2):
                    p = 32 * (2 * hi + h2)
                    nc.tensor.matmul(
                        ps_rb[64 * hi : 64 * hi + 64, :],
                        lhsT=ones_sb[p : p + 1, :],
                        rhs=recip[p : p + 1, :],
                        start=True,
                        stop=True,
                        tile_position=(p, 64 * hi),
                    )
                nc.vector.tensor_copy(rbc[:, h2 * 512 : (h2 + 1) * 512], ps_rb)
            for h2 in range(2):
                h2s = slice(h2 * 512, (h2 + 1) * 512)
                nc.vector.tensor_mul(ocat[:, h2s], ps_av[h2], rbc[:, h2s])

        def oproj_unit(nt):
            nts = slice(nt * 128, (nt + 1) * 128)
            ot = at.tile([128, 1024], F16, tag="ot", bufs=4, name="ot")
            for h2 in range(2):
                h2s = slice(h2 * 512, (h2 + 1) * 512)
                ps_u = psum.tile([128, 512], F32, tag="fl", bufs=1, name="ps_u")
                nc.tensor.matmul(
                    ps_u, lhsT=wo_sb[:, nts], rhs=ocat[:, h2s],
                    start=True, stop=True,
                )
                nc.vector.tensor_copy(ot[:, h2s], ps_u)
            nc.sync.dma_start(out=outTA[nts, qoff : qoff + QCH], in_=ot)

        return [norm_unit] + [lambda nt=nt: oproj_unit(nt) for nt in range(8)]

    # ---------------- schedule ----------------
    # Prologue: K-proj+rope and Q-proj+rope of t4=0 only; V of t4=0 runs as
    # chunk 0's first fills (PE does V-proj while DVE ropes Q). norm(c) must
    # be the first fill of chunk c+1, BEFORE its av(0): av(0) waits on the
    # av-ring freed by norm's DVE ops (FIFO engines deadlock otherwise).
    p0 = phase_units(0)
    p1 = phase_units(1)
    p2 = phase_units(2)
    p3 = phase_units(3)
    p0[0]()  # K proj half 0
    p0[1]()  # K proj half 1 + rope
    p0[4]()  # Q proj half 0
    p0[5]()  # Q proj half 1 + rope
    n00 = chunk(0, 0, [p0[2], p0[3]] + p1, pre_av=2)
    n01 = chunk(0, 1, [n00[0]] + p2 + n00[1:5])
    n10 = chunk(1, 0, [n01[0]] + p3 + n00[5:9])
    n11 = chunk(1, 1, [n10[0]] + n01[1:9] + n10[1:7])
    for u in n10[7:9] + n11:
        u()

    at.release()
    const.release()
    psum.release()


_NC_CACHE = {}


def _build_program():
    if 0 in _NC_CACHE:
        return _NC_CACHE[0]
    nc = bacc.Bacc("TRN2", num_devices=N_CORES, debug=False)
    x8 = nc.dram_tensor("xT", [D, NTOK], F16, kind="ExternalInput").ap()
    wq8 = nc.dram_tensor("wqT", [D, 128], F16, kind="ExternalInput").ap()
    wk8 = nc.dram_tensor("wkT", [D, 128], F16, kind="ExternalInput").ap()
    wv8 = nc.dram_tensor("wvT", [D, 128], F16, kind="ExternalInput").ap()
    woT = nc.dram_tensor("woT", [128, D], F16, kind="ExternalInput").ap()
    ropeA = nc.dram_tensor("ropeA", [128, NTOK], F16, kind="ExternalInput").ap()
    ropeB = nc.dram_tensor("ropeB", [128, NTOK], F16, kind="ExternalInput").ap()
    outTA = nc.dram_tensor("outTA", [D, NTOK], F16, kind="ExternalOutput").ap()
    with tile.TileContext(nc) as tc:
        _build_body(tc, x8, wq8, wk8, wv8, woT, ropeA, ropeB, outTA)
    nc.compile()
    _NC_CACHE[0] = nc
    return nc


def _rope_perm():
    """Within-head row permutation: new row 32q+j holds old dim 16q+j
    (x1, j<16) or 32+16q+(j-16) (x2, j>=16) so rotate-half partners sit
    +-16 apart inside one 32-partition quadrant."""
    perm = np.empty(DK, np.int64)
    for q in range(2):
        for j in range(32):
            r = 32 * q + j
            perm[r] = 16 * q + j if j < 16 else 32 + 16 * q + (j - 16)
    return perm  # new_row -> old_dim


def _rope_tables():
    half = DK // 2  # 32
    inv_freq = 1.0 / (
        10000.0 ** (np.arange(0, DK, 2, dtype=np.float32) / np.float32(DK))
    )
    t = np.arange(T, dtype=np.float32)
    freqs = np.outer(t, inv_freq)  # [T, 32]
    cos = np.cos(freqs)
    sin = np.sin(freqs)
    perm = _rope_perm()
    A = np.empty((128, NTOK), np.float32)
    Bt = np.empty((128, NTOK), np.float32)
    for p in range(128):
        d = perm[p % DK]
        if d < half:
            a, bb = cos[:, d], -sin[:, d]
        else:
            a, bb = cos[:, d - half], sin[:, d - half]
        for bi in range(B):
            A[p, bi * T : (bi + 1) * T] = a
            Bt[p, bi * T : (bi + 1) * T] = bb
    return A.astype(np.float16), Bt.astype(np.float16)


def _prep_inputs(x, wq, wk, wv, wo):
    xT = np.ascontiguousarray(x.reshape(NTOK, D).T).astype(np.float16)
    ropeA, ropeB = _rope_tables()
    perm = _rope_perm()
    full_perm = np.concatenate([64 * h + perm for h in range(2)])  # 128 rows
    in_maps = []
    for c in range(N_CORES):
        rows = slice(128 * c, 128 * (c + 1))
        wq_c = wq[rows, :][full_perm, :]
        wk_c = wk[rows, :][full_perm, :]
        in_maps.append(
            {
                "xT": xT,
                "wqT": np.ascontiguousarray(wq_c.T).astype(np.float16),
                "wkT": np.ascontiguousarray(wk_c.T).astype(np.float16),
                "wvT": np.ascontiguousarray(wv[rows, :].T).astype(np.float16),
                "woT": np.ascontiguousarray(wo[:, rows].T).astype(np.float16),
                "ropeA": ropeA,
                "ropeB": ropeB,
            }
        )
    return in_maps


def run(x, wq, wk, wv, wo, trace=False):
    """Returns (output (B,T,D) fp32, BassKernelResults)."""
    from concourse import bass_utils

    nc = _build_program()
    in_maps = _prep_inputs(
        np.asarray(x, np.float32),
        np.asarray(wq, np.float32),
        np.asarray(wk, np.float32),
        np.asarray(wv, np.float32),
        np.asarray(wo, np.float32),
    )
    res = bass_utils.run_bass_kernel_spmd(
        nc, in_maps, core_ids=list(range(N_CORES)), trace=trace
    )
    acc = np.zeros((D, NTOK), np.float32)
    for c in range(N_CORES):
        acc += np.asarray(res.results[c]["outTA"], np.float32)
    out = acc.T.reshape(B, T, D)
    return out, res


def kernel(x, wq, wk, wv, wo):
    out, _ = run(x, wq, wk, wv, wo)
    return out


# revision 8
# speedup vs baseline: 1.4401x; 1.0851x over previous
"""Multi-head attention (RoPE) Trainium2 kernel — v2.

Problem: B=2, T=2048, D_MODEL=1024, 16 heads x d_k=64, fp32 in/out.

Sharding: tensor-parallel over heads. Core c owns heads 2c, 2c+1:
  - wq/wk/wv rows [128c, 128c+128)  (column-split of the projections)
  - wo columns [128c, 128c+128)     (row-split of the output projection)
Each core emits a fully-normalized fp16 partial of the output projection;
the host sums the 8 partials (the "all-reduce" of row-parallel wo).

v2 changes vs the 301us baseline (trace-driven):
  - RoPE pair layout permuted host-side so the rotate-half partner of every
    row lives in the same 32-partition quadrant: the partition swap becomes
    one DVE stream_shuffle instead of 4 latency-critical SBUF-SBUF DMAs.
    (Scores are invariant to a common within-head permutation of Q/K rows.)
  - V transpose moved off the PE onto the DMA XBAR (dma_start_transpose,
    [128,128] f16 tiles) - frees ~12us of PE timeline + 64 DVE copies.
  - softmax reciprocal via reciprocal_approx_fast (5x) + f16 cast; the
    partition-broadcast stays on the PE (4-way concurrent M=1 matmuls
    measured at ~57ns/MM).
  - all x/rope-table tranches DMA'd up front (the sync queue no longer
    carries rope swaps, so nothing latency-critical queues behind them).
  - ACT exp table preloaded via a dummy activation at kernel start (hides
    the ~2.7us ACT_TABLE_LOAD under the prologue DMA wait).
  - V projection of t4=0 runs as the first fills of chunk 0 (PE does V-proj
    while DVE ropes Q), prologue = K-proj+rope, Q-proj+rope only.

Schedule: per kt slot the PE runs 4 score MMs (M=128, serial, ~0.9us) and
4 AV MMs + 4 ones MMs (M=64 pairs + 4-way M=1, ~0.9us) against an ACT
envelope of 2x 1147ns exps; fill units (proj halves / oproj / norm) are
interleaved one per slot into the remaining slack, deadline-ordered so no
matmul heads the PE queue before its producer is emitted.

PSUM (8 banks): big ring [128,1024]x2 = 4 (scores/proj/oproj/recip-bcast),
av [128,512]x2 = 2, ones [128,512]x2 = 2.
"""

import sys

sys.path.insert(0, "/opt/trn_rl_repo")

import numpy as np

import concourse.bacc as bacc
import concourse.bass as bass
import concourse.tile as tile
from concourse import mybir

F16 = mybir.dt.float16
F32 = mybir.dt.float32

B = 2
T = 2048
D = 1024
NTOK = B * T  # 4096
DK = 64
N_CORES = 8
QCH = 1024  # query chunk (per (b, qh))
KT_N = T // 128  # 16 key tiles per batch

SWAP_MASK = list(range(16, 32)) + list(range(0, 16))


def _build_body(tc, x8, wq8, wk8, wv8, woT, ropeA, ropeB, outTA):
    nc = tc.nc
    Exp = mybir.ActivationFunctionType.Exp

    const = tc.alloc_tile_pool(name="const", bufs=1)
    psum = tc.alloc_tile_pool(name="psum", bufs=1, space="PSUM")

    # ---------------- persistent tiles ----------------
    # DMA order = consumption order: wk (first PE work), x/rope t4=0, wq,
    # then the remaining tranches up front (nothing latency-critical queues
    # behind them anymore), wv, wo.
    w_sb = {}
    for nm, w in (("wk", wk8), ("wq", wq8), ("wv", wv8)):
        wt = const.tile([128, 8, 128], F16, name=f"{nm}sb")
        w_sb[nm] = wt

    xs = const.tile([128, 8, 4096], F16, name="xs")
    rA = const.tile([128, 4096], F16)
    rB = const.tile([128, 4096], F16)
    wo_sb = const.tile([128, 1024], F16)

    nc.sync.dma_start(
        out=w_sb["wk"], in_=wk8.rearrange("(a p) m -> p a m", p=128)
    )

    def load_t4(t4):
        cs = slice(t4 * 1024, (t4 + 1) * 1024)
        nc.sync.dma_start(
            out=xs[:, :, cs],
            in_=x8[:, cs].rearrange("(a p) n -> p a n", p=128),
        )
        nc.sync.dma_start(out=rA[:, cs], in_=ropeA[:, cs])
        nc.sync.dma_start(out=rB[:, cs], in_=ropeB[:, cs])

    load_t4(0)
    nc.sync.dma_start(
        out=w_sb["wq"], in_=wq8.rearrange("(a p) m -> p a m", p=128)
    )
    load_t4(1)
    nc.sync.dma_start(
        out=w_sb["wv"], in_=wv8.rearrange("(a p) m -> p a m", p=128)
    )
    load_t4(2)
    nc.sync.dma_start(out=wo_sb, in_=woT)
    load_t4(3)

    ones_sb = const.tile([128, 64], F16)
    nc.vector.memset(ones_sb, 1.0)

    q_rot = const.tile([128, 4096], F16)
    k_rot = const.tile([128, 4096], F16)
    vt_raw = const.tile([128, 4096], F16)
    # per 128-token tile: [tok, d'] fp16, both heads (A cols 0:64, B 64:128)
    v_sb = [const.tile([128, 128], F16, name=f"vsb{i}") for i in range(NTOK // 128)]

    at = tc.alloc_tile_pool(name="attn", bufs=1)

    # preload the exp table set under the prologue DMA wait
    warm = at.tile([128, 1], F16, tag="warm", bufs=1)
    nc.scalar.activation(warm, ones_sb[:, 0:1], Exp, scale=0.125)

    # ---------------- fill-unit building blocks ----------------
    # fills use a dedicated 1-bank psum ring so they never wait on the
    # ACT-paced scores ring
    def proj_half(nm, dst, t4, h2):
        off = t4 * 1024 + h2 * 512
        wt = w_sb[nm]
        ps = psum.tile([128, 512], F32, tag="fl", bufs=1, name="ps_pr")
        for k in range(8):
            nc.tensor.matmul(
                ps,
                lhsT=wt[:, k, :],
                rhs=xs[:, k, off : off + 512],
                start=(k == 0),
                stop=(k == 7),
            )
        nc.vector.tensor_copy(dst[:, off : off + 512], ps)
        if nm == "wv":
            # V^T -> per-token-tile [tok, d'] via the DMA XBAR transpose
            for i in range(off // 128, off // 128 + 4):
                ts = slice(i * 128, (i + 1) * 128)
                nc.sync.dma_start_transpose(out=v_sb[i], in_=vt_raw[:, ts])

    def rope_unit(raw, t4):
        # out = raw*A + shuffle(raw)*B; the rotate-half partner of every row
        # is +-16 within its 32-partition quadrant (host-permuted layout)
        cs = slice(t4 * 1024, (t4 + 1) * 1024)
        sw = at.tile([128, 1024], F16, tag="sw", bufs=2, name="ropesw")
        nc.vector.stream_shuffle(sw, raw[:, cs], SWAP_MASK)
        t1 = at.tile([128, 1024], F16, tag="t1", bufs=2, name="ropet1")
        nc.vector.tensor_mul(t1, raw[:, cs], rA[:, cs])
        nc.vector.tensor_mul(sw, sw, rB[:, cs])
        nc.vector.tensor_add(raw[:, cs], t1, sw)

    def phase_units(t4):
        # k first (attention kt tiles need it earliest), then v + transposes,
        # q last (only needed once this t4's own query chunk starts).
        return [
            lambda: proj_half("wk", k_rot, t4, 0),
            lambda: (proj_half("wk", k_rot, t4, 1), rope_unit(k_rot, t4)),
            lambda: proj_half("wv", vt_raw, t4, 0),
            lambda: proj_half("wv", vt_raw, t4, 1),
            lambda: proj_half("wq", q_rot, t4, 0),
            lambda: (proj_half("wq", q_rot, t4, 1), rope_unit(q_rot, t4)),
        ]

    # ---------------- attention ----------------
    def chunk(b, qh, fills, pre_av=1):
        """fills: callables, ~0.5-2us of PE work each, one per kt slot.
        pre_av: fills to run before av(0) (C00 needs its own V ready)."""
        qoff = b * T + qh * QCH
        fills = list(fills)

        exp_tiles = {}

        def s_exp(kt):
            # hi-major: the hi tile's 2 MMs run back-to-back so ACT's exp of
            # head hi starts after 2 MMs, and the per-hi ring slot frees
            # independently (PE's hi0 MMs of kt overlap ACT's hi1 of kt-1).
            koff = b * T + kt * 128
            for hi in range(2):
                ps = psum.tile([128, 1024], F32, tag="sc", bufs=2, name="ps_s")
                hs = slice(64 * hi, 64 * hi + 64)
                for h2 in range(2):
                    nc.tensor.matmul(
                        ps[:, h2 * 512 : (h2 + 1) * 512],
                        lhsT=k_rot[hs, koff : koff + 128],
                        rhs=q_rot[hs, qoff + h2 * 512 : qoff + (h2 + 1) * 512],
                        start=True,
                        stop=True,
                    )
                e = at.tile([128, 1024], F16, tag="exp", bufs=8, name="exps")
                nc.scalar.activation(e, ps, Exp, scale=0.125)
                exp_tiles[(hi, kt)] = e

        ps_av = []
        ps_ones = [None]

        def av(kt):
            vt = v_sb[b * KT_N + kt]
            eA = exp_tiles.pop((0, kt))
            eB = exp_tiles.pop((1, kt))
            st = dict(start=(kt == 0), stop=(kt == KT_N - 1), skip_group_check=True)
            for h2 in range(2):
                h2s = slice(h2 * 512, (h2 + 1) * 512)
                nc.tensor.matmul(
                    ps_av[h2][0:64, :], lhsT=vt[:, 0:64], rhs=eA[:, h2s], **st
                )
                nc.tensor.matmul(
                    ps_av[h2][64:128, :], lhsT=vt[:, 64:128], rhs=eB[:, h2s], **st
                )
            for hi in range(2):
                for h2 in range(2):
                    p = 32 * (2 * hi + h2)
                    e = eA if hi == 0 else eB
                    h2s = slice(h2 * 512, (h2 + 1) * 512)
                    nc.tensor.matmul(
                        ps_ones[0][p : p + 1, :],
                        lhsT=ones_sb[:, 0:1],
                        rhs=e[:, h2s],
                        tile_position=(0, p),
                        **st,
                    )

        # pipelined emission with TWO-slot av lag (exp tiles always banked
        # when av issues -> no LDW stalls on ACT); one fill unit per kt slot
        s_exp(0)
        ps_av.extend(
            psum.tile([128, 512], F32, tag="av", bufs=2, name=f"ps_av{h2}")
            for h2 in range(2)
        )
        ps_ones[0] = psum.tile([128, 512], F32, tag="ones", bufs=1, name="ps_on")
        s_exp(1)
        if fills:
            fills.pop(0)()
        s_exp(2)
        for _ in range(max(0, pre_av - 1)):
            if fills:
                fills.pop(0)()
        av(0)
        for kt in range(3, KT_N):
            s_exp(kt)
            av(kt - 2)
            if fills:
                fills.pop(0)()
        av(KT_N - 2)
        if fills:
            fills.pop(0)()
        av(KT_N - 1)
        while fills:
            fills.pop(0)()

        ocat = at.tile([128, 1024], F16, tag="ocat", bufs=3, name="ocat")

        def norm_unit():
            # reciprocal of the 4 denominator rows (fast-approx, ~5x), cast
            # to f16, partition-broadcast via K=1 outer-product matmuls
            # (4-way concurrent), then ocat = O_unnorm * recip.
            r32 = at.tile([128, 512], F32, tag="r32", bufs=2, name="r32")
            nc.vector.reciprocal_approx_fast(r32, ps_ones[0])
            recip = at.tile([128, 512], F16, tag="recip", bufs=2, name="recip")
            nc.vector.tensor_copy(recip, r32)
            rbc = at.tile([128, 1024], F16, tag="rbc", bufs=2, name="rbc")
            for h2 in range(2):
                ps_rb = psum.tile([128, 512], F32, tag="fl", bufs=1, name="ps_rb")
                for hi in range(2):
                    p = 32 * (2 * hi + h2)
                    nc.tensor.matmul(
                        ps_rb[64 * hi : 64 * hi + 64, :],
                        lhsT=ones_sb[p : p + 1, :],
                        rhs=recip[p : p + 1, :],
                        start=True,
                        stop=True,
                        tile_position=(p, 64 * hi),
                    )
                nc.vector.tensor_copy(rbc[:, h2 * 512 : (h2 + 1) * 512], ps_rb)
            for h2 in range(2):
                h2s = slice(h2 * 512, (h2 + 1) * 512)
                nc.vector.tensor_mul(ocat[:, h2s], ps_av[h2], rbc[:, h2s])

        def oproj_unit(nt):
            # rides the sc ring's natural third rotation slot between ACT
            # reads (prompt mid-slot) instead of the 1-deep fills bank
            nts = slice(nt * 128, (nt + 1) * 128)
            ps_u = psum.tile([128, 1024], F32, tag="sc", bufs=2, name="ps_u")
            for h2 in range(2):
                h2s = slice(h2 * 512, (h2 + 1) * 512)
                nc.tensor.matmul(
                    ps_u[:, h2s], lhsT=wo_sb[:, nts], rhs=ocat[:, h2s],
                    start=True, stop=True,
                )
            ot = at.tile([128, 1024], F16, tag="ot", bufs=4, name="ot")
            nc.vector.tensor_copy(ot, ps_u)
            nc.sync.dma_start(out=outTA[nts, qoff : qoff + QCH], in_=ot)

        return [norm_unit] + [lambda nt=nt: oproj_unit(nt) for nt in range(8)]

    # ---------------- schedule ----------------
    # Prologue: K-proj+rope and Q-proj+rope of t4=0 only; V of t4=0 runs as
    # chunk 0's first fills (PE does V-proj while DVE ropes Q). norm(c) must
    # be the first fill of chunk c+1, BEFORE its av(0): av(0) waits on the
    # av-ring freed by norm's DVE ops (FIFO engines deadlock otherwise).
    p0 = phase_units(0)
    p1 = phase_units(1)
    p2 = phase_units(2)
    p3 = phase_units(3)
    p0[0]()  # K proj half 0
    p0[1]()  # K proj half 1 + rope
    p0[4]()  # Q proj half 0
    p0[5]()  # Q proj half 1 + rope
    n00 = chunk(0, 0, [p0[2], p0[3]] + p1, pre_av=2)
    n01 = chunk(0, 1, [n00[0]] + p2 + n00[1:5])
    n10 = chunk(1, 0, [n01[0]] + p3 + n00[5:9])
    n11 = chunk(1, 1, [n10[0]] + n01[1:9] + n10[1:7])
    for u in n10[7:9] + n11:
        u()

    at.release()
    const.release()
    psum.release()


_NC_CACHE = {}


def _build_program():
    if 0 in _NC_CACHE:
        return _NC_CACHE[0]
    nc = bacc.Bacc("TRN2", num_devices=N_CORES, debug=False)
    x8 = nc.dram_tensor("xT", [D, NTOK], F16, kind="ExternalInput").ap()
    wq8 = nc.dram_tensor("wqT", [D, 128], F16, kind="ExternalInput").ap()
    wk8 = nc.dram_tensor("wkT", [D, 128], F16, kind="ExternalInput").ap()
    wv8 = nc.dram_tensor("wvT", [D, 128], F16, kind="ExternalInput").ap()
    woT = nc.dram_tensor("woT", [128, D], F16, kind="ExternalInput").ap()
    ropeA = nc.dram_tensor("ropeA", [128, NTOK], F16, kind="ExternalInput").ap()
    ropeB = nc.dram_tensor("ropeB", [128, NTOK], F16, kind="ExternalInput").ap()
    outTA = nc.dram_tensor("outTA", [D, NTOK], F16, kind="ExternalOutput").ap()
    with tile.TileContext(nc) as tc:
        _build_body(tc, x8, wq8, wk8, wv8, woT, ropeA, ropeB, outTA)
    nc.compile()
    _NC_CACHE[0] = nc
    return nc


def _rope_perm():
    """Within-head row permutation: new row 32q+j holds old dim 16q+j
    (x1, j<16) or 32+16q+(j-16) (x2, j>=16) so rotate-half partners sit
    +-16 apart inside one 32-partition quadrant."""
    perm = np.empty(DK, np.int64)
    for q in range(2):
        for j in range(32):
            r = 32 * q + j
            perm[r] = 16 * q + j if j < 16 else 32 + 16 * q + (j - 16)
    return perm  # new_row -> old_dim


def _rope_tables():
    half = DK // 2  # 32
    inv_freq = 1.0 / (
        10000.0 ** (np.arange(0, DK, 2, dtype=np.float32) / np.float32(DK))
    )
    t = np.arange(T, dtype=np.float32)
    freqs = np.outer(t, inv_freq)  # [T, 32]
    cos = np.cos(freqs)
    sin = np.sin(freqs)
    perm = _rope_perm()
    A = np.empty((128, NTOK), np.float32)
    Bt = np.empty((128, NTOK), np.float32)
    for p in range(128):
        d = perm[p % DK]
        if d < half:
            a, bb = cos[:, d], -sin[:, d]
        else:
            a, bb = cos[:, d - half], sin[:, d - half]
        for bi in range(B):
            A[p, bi * T : (bi + 1) * T] = a
            Bt[p, bi * T : (bi + 1) * T] = bb
    return A.astype(np.float16), Bt.astype(np.float16)


def _prep_inputs(x, wq, wk, wv, wo):
    xT = np.ascontiguousarray(x.reshape(NTOK, D).T).astype(np.float16)
    ropeA, ropeB = _rope_tables()
    perm = _rope_perm()
    full_perm = np.concatenate([64 * h + perm for h in range(2)])  # 128 rows
    in_maps = []
    for c in range(N_CORES):
        rows = slice(128 * c, 128 * (c + 1))
        wq_c = wq[rows, :][full_perm, :]
        wk_c = wk[rows, :][full_perm, :]
        in_maps.append(
            {
                "xT": xT,
                "wqT": np.ascontiguousarray(wq_c.T).astype(np.float16),
                "wkT": np.ascontiguousarray(wk_c.T).astype(np.float16),
                "wvT": np.ascontiguousarray(wv[rows, :].T).astype(np.float16),
                "woT": np.ascontiguousarray(wo[:, rows].T).astype(np.float16),
                "ropeA": ropeA,
                "ropeB": ropeB,
            }
        )
    return in_maps


def run(x, wq, wk, wv, wo, trace=False):
    """Returns (output (B,T,D) fp32, BassKernelResults)."""
    from concourse import bass_utils

    nc = _build_program()
    in_maps = _prep_inputs(
        np.asarray(x, np.float32),
        np.asarray(wq, np.float32),
        np.asarray(wk, np.float32),
        np.asarray(wv, np.float32),
        np.asarray(wo, np.float32),
    )
    res = bass_utils.run_bass_kernel_spmd(
        nc, in_maps, core_ids=list(range(N_CORES)), trace=trace
    )
    acc = np.zeros((D, NTOK), np.float32)
    for c in range(N_CORES):
        acc += np.asarray(res.results[c]["outTA"], np.float32)
    out = acc.T.reshape(B, T, D)
    return out, res


def kernel(x, wq, wk, wv, wo):
    out, _ = run(x, wq, wk, wv, wo)
    return out
